# revision 1
# baseline (speedup 1.0000x reference)
"""Trainium2 Bass kernel for nn_Encoder_71528385347709 (gnn_message_passing).

3-layer TransformerConv (heads=1) GNN encoder + per-layer global mean pool.

Distribution: nodes sharded graph-contiguously across 8 NeuronCores (8 graphs
each, batch_ids sorted); edges assigned to the dst-owning core; per-layer halo
exchange of node states via a shared-output AllGather.  Per-edge work reduces
to a single gather of the source-node state h_src (512B rows) through
    alpha_e = (q @ Wk^T)_dst . h_src  (+ per-dst const, softmax-invariant)
    agg_dst = (sum_e softmax(alpha)_e h_src_e) @ Wv + bv
so no per-edge k/v is ever materialized.  Gathers use dma_gather (int16
indices) over the two halves of the replicated node table.
"""
import sys
import numpy as np

sys.path.insert(0, '/opt/trn_rl_repo')

import concourse.bass as bass              # noqa: E402
import concourse.tile as tile              # noqa: E402
from concourse import bacc, mybir          # noqa: E402
from concourse.masks import make_identity  # noqa: E402
import concourse.bass_utils as bass_utils  # noqa: E402

F32 = mybir.dt.float32
BF16 = mybir.dt.bfloat16
I16 = mybir.dt.int16
ALU = mybir.AluOpType
AXL = mybir.AxisListType
ACTF = mybir.ActivationFunctionType

NCORES = 8
C = 128
DEPTH = 3
B_GRAPHS = 64
GCOLS = 32          # max slot-grid columns resident per group
CALLCOLS = 8        # slot-grid columns per dma_gather call (NI <= 1024)
NEG = -1.0e30


# ---------------------------------------------------------------- host prep
def preprocess(edge_index, batch_ids, n_graphs=B_GRAPHS):
    src = np.asarray(edge_index[0], np.int64)
    dst = np.asarray(edge_index[1], np.int64)
    bid = np.asarray(batch_ids, np.int64)
    N = bid.shape[0]
    Etot = src.shape[0]
    gpc = n_graphs // NCORES

    bounds = np.searchsorted(bid, np.arange(NCORES + 1) * gpc)
    L = np.diff(bounds)
    NB = int(np.ceil((L.max() + 1) / 128.0))
    PL = NB * 128
    NF = NCORES * PL
    A_HI = min(NF, 32768)
    B_LO = max(0, NF - 32768)
    assert NF - B_LO <= 32768

    indeg = np.bincount(dst, minlength=N)
    dev_row = np.empty(N, np.int64)
    perms = []
    for c in range(NCORES):
        n0, n1 = int(bounds[c]), int(bounds[c + 1])
        order = np.argsort(indeg[n0:n1], kind='stable')
        perm = n0 + order
        perms.append(perm)
        dev_row[perm] = c * PL + np.arange(n1 - n0)

    sdev = dev_row[src]
    ddev = dev_row[dst]
    # balanced assignment: src < B_LO must be A, >= A_HI must be B, the
    # overlap window is assigned per-dst to balance list lengths
    mustA = sdev < B_LO
    flex = (~mustA) & (sdev < A_HI)
    cAm = np.bincount(ddev, weights=mustA.astype(np.float64), minlength=NF).astype(np.int64)
    nflex = np.bincount(ddev, weights=flex.astype(np.float64), minlength=NF).astype(np.int64)
    tot = np.bincount(ddev, minlength=NF).astype(np.int64)
    dA_t = np.clip((tot + 1) // 2, cAm, cAm + nflex)
    # rank of each flex edge within its dst's flex list
    keyf = ddev * 2 + (~flex)
    eof = np.argsort(keyf, kind='stable')
    fstart = np.searchsorted(keyf[eof], np.arange(NF) * 2)
    frank = np.empty(Etot, np.int64)
    frank[eof] = np.arange(Etot) - fstart[ddev[eof]]
    isA = mustA | (flex & (frank < (dA_t - cAm)[ddev]))

    cA = np.bincount(ddev, weights=isA.astype(np.float64), minlength=NF)
    cB = np.bincount(ddev, weights=(~isA).astype(np.float64), minlength=NF)
    cA = cA.astype(np.int64).reshape(NCORES, NB, 128)
    cB = cB.astype(np.int64).reshape(NCORES, NB, 128)
    DA = cA.max(axis=(0, 2))
    DB = cB.max(axis=(0, 2))
    assert int((DA + DB).max()) <= GCOLS, f"block cols {int((DA+DB).max())} > GCOLS"

    groups, cur, cur_cols = [], [], 0
    for b in range(NB):
        w = int(DA[b] + DB[b])
        if cur and cur_cols + w > GCOLS:
            groups.append(cur)
            cur, cur_cols = [], 0
        cur.append(b)
        cur_cols += w
    if cur:
        groups.append(cur)

    ginfo = []
    colA_base = np.zeros(NB, np.int64)
    colB_base = np.zeros(NB, np.int64)
    S_total = 0
    for blocks in groups:
        g = {"blocks": blocks, "col0": S_total}
        off = 0
        for b in blocks:
            colA_base[b] = S_total + off
            off += int(DA[b])
        g["ka"] = off
        for b in blocks:
            colB_base[b] = S_total + off
            off += int(DB[b])
        g["kb"] = off - g["ka"]
        g["cols"] = off
        S_total += off
        ginfo.append(g)

    dummyA = PL - 1
    dummyB = NF - 1 - B_LO
    colIsB = np.zeros(S_total, bool)
    for g in ginfo:
        colIsB[g["col0"] + g["ka"]: g["col0"] + g["cols"]] = True

    sgrid = np.where(colIsB[None, None, :], dummyB, dummyA) * np.ones(
        (NCORES, 128, 1), np.int64)
    maskg = np.full((NCORES, 128, S_total), NEG, np.float32)

    key = ddev * 2 + (~isA)
    eo = np.argsort(key, kind='stable')
    k_sorted = key[eo]
    sdev_o = sdev[eo]
    rowstartA = np.searchsorted(k_sorted, np.arange(NF) * 2)
    rowstartB = np.searchsorted(k_sorted, np.arange(NF) * 2 + 1)
    r = ddev[eo]
    c_ = r // PL
    jloc = r % PL
    b_ = jloc // 128
    p_ = jloc % 128
    e_isA = isA[eo]
    ar = np.arange(Etot)
    rankA = ar - rowstartA[r]
    rankB = ar - rowstartB[r]
    col = np.where(e_isA, colA_base[b_] + rankA, colB_base[b_] + rankB)
    val = np.where(e_isA, sdev_o, sdev_o - B_LO)
    sgrid[c_, p_, col] = val
    maskg[c_, p_, col] = 0.0
    assert sgrid.max() <= 32767 and sgrid.min() >= 0

    calls = []
    for g in ginfo:
        c0 = g["col0"]
        for a0 in range(0, g["ka"], CALLCOLS):
            calls.append((c0 + a0, min(CALLCOLS, g["ka"] - a0), False))
        for bb0 in range(0, g["kb"], CALLCOLS):
            calls.append((c0 + g["ka"] + bb0, min(CALLCOLS, g["kb"] - bb0), True))

    idx_cols = sum(8 * n for (_, n, _) in calls)
    idx16 = np.zeros((NCORES, 128, idx_cols), np.int16)
    off = 0
    call_meta = []
    for (cs, n, is_b) in calls:
        ni = 128 * n
        flat = sgrid[:, :, cs:cs + n].transpose(0, 2, 1).reshape(NCORES, ni)
        wrapped = flat.reshape(NCORES, ni // 16, 16).transpose(0, 2, 1)
        idx16[:, :, off:off + ni // 16] = np.tile(wrapped, (1, 8, 1)).astype(np.int16)
        call_meta.append({"grid_col": cs, "ncols": n, "is_b": is_b,
                          "idx_off": off, "ni": ni})
        off += ni // 16

    aux = np.zeros((NCORES, 128, NB * 10), np.float32)
    for c in range(NCORES):
        Lc = int(L[c])
        gid = bid[perms[c]] - c * gpc
        j = np.arange(Lc)
        aux[c, j % 128, (j // 128) * 10 + gid] = 1.0
        aux[c, j % 128, (j // 128) * 10 + 8] = 1.0
        aux[c, j % 128, (j // 128) * 10 + 9] = (indeg[perms[c]] > 0).astype(np.float32)

    aux2 = np.zeros((NCORES, 2, PL), np.float32)
    for c in range(NCORES):
        Lc = int(L[c])
        aux2[c, 0, :Lc] = (indeg[perms[c]] > 0).astype(np.float32)
        aux2[c, 1, :] = 1.0
    counts = np.maximum(np.bincount(bid, minlength=n_graphs), 1).astype(np.float32)
    invc = (1.0 / counts).reshape(NCORES, gpc, 1)

    return dict(NB=NB, PL=PL, NF=NF, B_LO=B_LO, aux2=aux2, groups=ginfo,
                DA=DA, DB=DB, colA_base=colA_base, colB_base=colB_base,
                call_meta=call_meta, idx16=idx16, maskg=maskg, aux=aux,
                invc=invc, perms=perms, L=L, S_total=S_total, gpc=gpc)


# ---------------------------------------------------------------- device build
def build(meta, reps=1, gather_from_shared=True, scratch=16384, nq=4,
          skip_gather=False, skip_edgedve=False, skip_exchange=False,
          fused_update=False, grouped_edge=False):
    NB, PL, NF, B_LO = meta["NB"], meta["PL"], meta["NF"], meta["B_LO"]
    S_total = meta["S_total"]
    idx_cols = meta["idx16"].shape[2]
    gpc = meta["gpc"]
    scale = float(1.0 / np.sqrt(C))

    nc = bacc.Bacc("TRN2", target_bir_lowering=False, debug=False,
                   num_devices=NCORES, dynamic_dma_scratch_size=scratch,
                   num_swdge_queues=nq)

    xT_d = nc.dram_tensor("xT", [4, PL], BF16, kind="ExternalInput")
    encW_d = nc.dram_tensor("encW", [4, C - 4], BF16, kind="ExternalInput")
    encbr_d = nc.dram_tensor("encbr", [128, C - 4], F32, kind="ExternalInput")
    Wq_d = nc.dram_tensor("Wq", [DEPTH, C, C], BF16, kind="ExternalInput")
    WkT_d = nc.dram_tensor("WkT", [DEPTH, C, C], BF16, kind="ExternalInput")
    Wv_d = nc.dram_tensor("Wv", [DEPTH, C, C], BF16, kind="ExternalInput")
    Ws_d = nc.dram_tensor("Ws", [DEPTH, C, C], BF16, kind="ExternalInput")
    bq_d = nc.dram_tensor("bq", [DEPTH, C, 1], F32, kind="ExternalInput")
    bvs_d = nc.dram_tensor("bvs", [2, DEPTH * C], BF16, kind="ExternalInput")
    aux2_d = nc.dram_tensor("aux2", [2, PL], BF16, kind="ExternalInput")
    idx_d = nc.dram_tensor("idx16", [128, idx_cols], I16, kind="ExternalInput")
    mask_d = nc.dram_tensor("maskg", [128, S_total], BF16, kind="ExternalInput")
    aux_d = nc.dram_tensor("aux", [128, NB * 10], BF16, kind="ExternalInput")
    auxf_d = nc.dram_tensor("auxf", [128, NB], F32, kind="ExternalInput")
    invc_d = nc.dram_tensor("invc", [gpc, 1], F32, kind="ExternalInput")
    out_d = nc.dram_tensor("out", [gpc, DEPTH * C], F32, kind="ExternalOutput")

    hf_sh = [nc.dram_tensor(f"hf{l}", [NF, C], BF16, addr_space="Shared")
             for l in range(DEPTH)]
    hf_loc = None if gather_from_shared else nc.dram_tensor("hfloc", [NF, C], BF16)
    in_b = nc.dram_tensor("in_b", [PL, C], BF16)

    lowprec = nc.allow_low_precision(reason="bf16 edge phase, tol 2e-2")
    with tile.TileContext(nc) as tc, lowprec:
        with tc.tile_pool(name="cst", bufs=1) as cst, \
             tc.tile_pool(name="st", bufs=1) as st, \
             tc.tile_pool(name="wk", bufs=2) as wk, \
             tc.tile_pool(name="wk1", bufs=1) as wk1, \
             tc.tile_pool(name="hgp", bufs=2) as hgp, \
             tc.tile_pool(name="ps", bufs=2, space="PSUM") as ps, \
             tc.tile_pool(name="ps1", bufs=1, space="PSUM") as ps1, \
             tc.tile_pool(name="psacc", bufs=1, space="PSUM") as psacc:

            # ---- constants (loaded once)
            xT = cst.tile([4, PL], BF16)
            nc.sync.dma_start(out=xT[:], in_=xT_d[:])
            encW = cst.tile([4, C - 4], BF16)
            encbr = cst.tile([128, C - 4], F32)
            nc.sync.dma_start(out=encW[:], in_=encW_d[:])
            nc.sync.dma_start(out=encbr[:], in_=encbr_d[:])
            Wq = cst.tile([C, DEPTH, C], BF16)
            WkT = cst.tile([C, DEPTH, C], BF16)
            Wv = cst.tile([C, DEPTH, C], BF16)
            Ws = cst.tile([C, DEPTH, C], BF16)
            bq = cst.tile([C, DEPTH, 1], F32)
            for (t, d) in ((Wq, Wq_d), (WkT, WkT_d), (Wv, Wv_d), (Ws, Ws_d),
                           (bq, bq_d)):
                nc.sync.dma_start(out=t[:], in_=d[:].rearrange("l a b -> a l b"))
            bvs = cst.tile([2, DEPTH, C], BF16)
            nc.sync.dma_start(out=bvs[:],
                              in_=bvs_d[:].rearrange("t (l c) -> t l c", l=DEPTH))
            aux2 = cst.tile([2, PL], BF16)
            nc.sync.dma_start(out=aux2[:], in_=aux2_d[:])
            idx16 = cst.tile([128, idx_cols], I16)
            nc.sync.dma_start(out=idx16[:], in_=idx_d[:])
            maskg = cst.tile([128, S_total], BF16)
            nc.sync.dma_start(out=maskg[:], in_=mask_d[:])
            aux = cst.tile([128, NB, 10], BF16)
            nc.sync.dma_start(out=aux[:],
                              in_=aux_d[:].rearrange("p (b t) -> p b t", b=NB))
            auxf = cst.tile([128, NB, 1], F32)
            nc.sync.dma_start(out=auxf[:].rearrange("p b o -> p (b o)"),
                              in_=auxf_d[:])
            invc = cst.tile([gpc, 1], F32)
            nc.sync.dma_start(out=invc[:], in_=invc_d[:])
            ident = cst.tile([128, 128], BF16)
            make_identity(nc, ident[:])

            # ---- persistent state
            hT = st.tile([128, PL], BF16)
            hnm = st.tile([128, NB, C], BF16)
            qtld = st.tile([128, NB, C], BF16)
            outp = st.tile([gpc, DEPTH * C], F32)

            for _rep in range(reps):
                # ===== h0 = [x, x@encW + encb], node-major, then transpose
                for b in range(NB):
                    pb = ps.tile([128, C], F32, space="PSUM", tag="pb")
                    nc.tensor.matmul(out=pb[:, 0:C - 4],
                                     lhsT=xT[:, b * 128:(b + 1) * 128],
                                     rhs=encW[:], start=True, stop=True)
                    nc.vector.tensor_tensor(
                        out=hnm[:, b, 4:C], in0=pb[:, 0:C - 4],
                        in1=encbr[:], op=ALU.add)
                    # first 4 channels: copy x rows (from xT via transpose trick:
                    # xT[:, block] is [4, 128]; transpose -> [128, 4])
                    ptr = ps.tile([128, 128], BF16, space="PSUM", tag="ptr")
                    nc.tensor.transpose(out=ptr[:, 0:4],
                                        in_=xT[:, b * 128:(b + 1) * 128],
                                        identity=ident[0:4, 0:4])
                    nc.vector.tensor_copy(out=hnm[:, b, 0:4], in_=ptr[:, 0:4])
                    nc.vector.tensor_scalar(out=hnm[:, b, :], in0=hnm[:, b, :],
                                            scalar1=auxf[:, b, 0:1], scalar2=None,
                                            op0=ALU.mult)
                    ptr2 = ps.tile([128, 128], BF16, space="PSUM", tag="ptr")
                    nc.tensor.transpose(out=ptr2[:], in_=hnm[:, b, :],
                                        identity=ident[:])
                    nc.scalar.copy(out=hT[:, b * 128:(b + 1) * 128],
                                   in_=ptr2[:])

                for l in range(DEPTH):
                    # ===== halo exchange
                    nc.sync.dma_start(
                        out=in_b[:].rearrange("(b p) c -> p b c", p=128),
                        in_=hnm[:])
                    if not skip_exchange:
                        nc.gpsimd.collective_compute(
                            "AllGather", ALU.bypass,
                            replica_groups=[list(range(NCORES))],
                            ins=[in_b[:].opt()], outs=[hf_sh[l][:].opt()])
                    if gather_from_shared:
                        hf = hf_sh[l]
                    else:
                        if not skip_exchange:
                            nc.sync.dma_start(out=hf_loc[:], in_=hf_sh[l][:])
                        hf = hf_loc

                    # ===== node phase: q, qtilde
                    for n0 in range(0, PL, 512):
                        nw = min(512, PL - n0)
                        pq = ps.tile([C, 512], F32, space="PSUM", tag="pqt")
                        nc.tensor.matmul(out=pq[:, 0:nw], lhsT=Wq[:, l, :],
                                         rhs=hT[:, n0:n0 + nw],
                                         start=True, stop=True)
                        qt = wk.tile([C, 512], BF16, tag="qt")
                        nc.vector.tensor_scalar(out=qt[:, 0:nw], in0=pq[:, 0:nw],
                                                scalar1=bq[:, l, 0:1],
                                                scalar2=None, op0=ALU.add)
                        for s in range(nw // 128):
                            b = n0 // 128 + s
                            pb = ps.tile([128, C], F32, space="PSUM", tag="pb")
                            nc.tensor.matmul(out=pb[:],
                                             lhsT=qt[:, s * 128:(s + 1) * 128],
                                             rhs=WkT[:, l, :],
                                             start=True, stop=True)
                            nc.vector.tensor_scalar(
                                out=qtld[:, b, :], in0=pb[:], scalar1=scale,
                                scalar2=None, op0=ALU.mult)

                    # ===== edge + update phase, grouped
                    ppool = psacc.tile([gpc, C], F32, space="PSUM", tag="pp")
                    qrr = 0
                    for g in meta["groups"]:
                        gc0, gcols = g["col0"], g["cols"]
                        hg = hgp.tile([128, GCOLS, C], BF16, tag="hg")
                        for cm in meta["call_meta"]:
                            if skip_gather:
                                break
                            if not (gc0 <= cm["grid_col"] < gc0 + gcols):
                                continue
                            lc0 = cm["grid_col"] - gc0
                            src_ap = hf[B_LO:, :] if cm["is_b"] else hf[:32768 if NF > 32768 else NF, :]
                            nc.gpsimd.dma_gather(
                                out_ap=hg[:, lc0:lc0 + cm["ncols"], :],
                                in_ap=src_ap,
                                idxs_ap=idx16[:, cm["idx_off"]:
                                              cm["idx_off"] + cm["ni"] // 16],
                                num_idxs=cm["ni"], num_idxs_reg=cm["ni"],
                                elem_size=C, queue_num=qrr % nq)
                            qrr += 1
                        use_grouped = grouped_edge and not skip_edgedve
                        for b in g["blocks"]:
                            da, db = int(meta["DA"][b]), int(meta["DB"][b])
                            dt = da + db
                            u_b = wk.tile([128, C], BF16, tag="ub")
                            if dt == 0 or skip_edgedve:
                                nc.vector.memset(u_b[:], 0.0)
                            else:
                                a0 = int(meta["colA_base"][b]) - gc0
                                b0 = int(meta["colB_base"][b]) - gc0
                                wh = wk1.tile([128, GCOLS, C], BF16, tag="wh")
                                alph = wk.tile([128, GCOLS], BF16, tag="al")
                                if da:
                                    nc.vector.tensor_tensor(
                                        out=wh[:, 0:da, :],
                                        in0=hg[:, a0:a0 + da, :],
                                        in1=qtld[:, b, :].unsqueeze(1)
                                        .to_broadcast([128, da, C]),
                                        op=ALU.mult)
                                if db:
                                    nc.vector.tensor_tensor(
                                        out=wh[:, da:dt, :],
                                        in0=hg[:, b0:b0 + db, :],
                                        in1=qtld[:, b, :].unsqueeze(1)
                                        .to_broadcast([128, db, C]),
                                        op=ALU.mult)
                                nc.vector.tensor_reduce(
                                    out=alph[:, 0:dt], in_=wh[:, 0:dt, :],
                                    axis=AXL.X, op=ALU.add)
                                if da:
                                    nc.vector.tensor_tensor(
                                        out=alph[:, 0:da], in0=alph[:, 0:da],
                                        in1=maskg[:, gc0 + a0:gc0 + a0 + da],
                                        op=ALU.add)
                                if db:
                                    nc.vector.tensor_tensor(
                                        out=alph[:, da:dt], in0=alph[:, da:dt],
                                        in1=maskg[:, gc0 + b0:gc0 + b0 + db],
                                        op=ALU.add)
                                nmax = wk.tile([128, 1], F32, tag="nm")
                                nc.vector.tensor_reduce(
                                    out=nmax[:], in_=alph[:, 0:dt],
                                    axis=AXL.X, op=ALU.max, negate=True)
                                ex = wk.tile([128, GCOLS], BF16, tag="ex")
                                ssum = wk.tile([128, 1], F32, tag="ss")
                                nc.scalar.activation(
                                    out=ex[:, 0:dt], in_=alph[:, 0:dt],
                                    func=ACTF.Exp, bias=nmax[:, 0:1],
                                    scale=1.0, accum_out=ssum[:])
                                rcp = wk.tile([128, 1], F32, tag="rc")
                                nc.vector.reciprocal(out=rcp[:], in_=ssum[:])
                                nc.vector.tensor_scalar(
                                    out=ex[:, 0:dt], in0=ex[:, 0:dt],
                                    scalar1=rcp[:, 0:1], scalar2=None,
                                    op0=ALU.mult)
                                if da:
                                    nc.vector.tensor_tensor(
                                        out=wh[:, 0:da, :],
                                        in0=hg[:, a0:a0 + da, :],
                                        in1=ex[:, 0:da].unsqueeze(2)
                                        .to_broadcast([128, da, C]),
                                        op=ALU.mult)
                                if db:
                                    nc.vector.tensor_tensor(
                                        out=wh[:, da:dt, :],
                                        in0=hg[:, b0:b0 + db, :],
                                        in1=ex[:, da:dt].unsqueeze(2)
                                        .to_broadcast([128, db, C]),
                                        op=ALU.mult)
                                m = dt
                                while m > 1:
                                    k = (m + 1) // 2
                                    h = m - k
                                    nc.vector.tensor_tensor(
                                        out=wh[:, 0:h, :], in0=wh[:, 0:h, :],
                                        in1=wh[:, k:k + h, :], op=ALU.add)
                                    m = k
                                nc.vector.tensor_copy(out=u_b[:], in_=wh[:, 0, :])
                            # --- per-block update: skip + bias + agg fused in PSUM
                            ptr = ps.tile([128, 128], BF16, space="PSUM", tag="ptr")
                            nc.tensor.transpose(out=ptr[:], in_=u_b[:],
                                                identity=ident[:])
                            uTb = wk.tile([128, 128], BF16, tag="uTb")
                            nc.scalar.copy(out=uTb[:], in_=ptr[:])
                            psk = ps.tile([128, C], F32, space="PSUM", tag="pb")
                            nc.tensor.matmul(out=psk[:],
                                             lhsT=hT[:, b * 128:(b + 1) * 128],
                                             rhs=Ws[:, l, :], start=True, stop=False)
                            nc.tensor.matmul(out=psk[:],
                                             lhsT=aux2[:, b * 128:(b + 1) * 128],
                                             rhs=bvs[:, l, :], start=False, stop=False)
                            nc.tensor.matmul(out=psk[:], lhsT=uTb[:],
                                             rhs=Wv[:, l, :], start=False, stop=True)
                            nc.vector.tensor_scalar(out=hnm[:, b, :], in0=psk[:],
                                                    scalar1=auxf[:, b, 0:1],
                                                    scalar2=None, op0=ALU.mult)
                            nc.tensor.matmul(out=ppool[:], lhsT=aux[:, b, 0:gpc],
                                             rhs=hnm[:, b, :], start=(b == 0),
                                             stop=(b == NB - 1))
                            ptr2 = ps.tile([128, 128], BF16, space="PSUM", tag="ptr")
                            nc.tensor.transpose(out=ptr2[:], in_=hnm[:, b, :],
                                                identity=ident[:])
                            nc.scalar.copy(
                                out=hT[:, b * 128:(b + 1) * 128], in_=ptr2[:])
                    nc.vector.tensor_scalar(out=outp[:, l * C:(l + 1) * C],
                                            in0=ppool[:], scalar1=invc[:, 0:1],
                                            scalar2=None, op0=ALU.mult)

            nc.sync.dma_start(out=out_d[:], in_=outp[:])
    nc.compile()
    return nc


# ---------------------------------------------------------------- input maps
def input_maps(meta, x, enc_W, enc_b, Wq, bq, Wk, bk, Wv, bv, Ws, bs):
    PL = meta["PL"]
    BF = mybir.dt.np(mybir.dt.bfloat16)
    in_maps = []
    WkT = np.ascontiguousarray(np.transpose(np.asarray(Wk, np.float32), (0, 2, 1)))
    bvs = np.stack([np.asarray(bv, np.float32),
                    np.asarray(bs, np.float32)], axis=1)
    for c in range(NCORES):
        perm = meta["perms"][c]
        Lc = int(meta["L"][c])
        xp = np.zeros((PL, 4), np.float32)
        xp[:Lc] = np.asarray(x, np.float32)[perm]
        in_maps.append({
            "xT": np.ascontiguousarray(xp.T).astype(BF),
            "encW": np.asarray(enc_W, np.float32).astype(BF),
            "encbr": np.tile(np.asarray(enc_b, np.float32).reshape(1, -1), (128, 1)),
            "Wq": np.asarray(Wq, np.float32).astype(BF),
            "WkT": WkT.astype(BF),
            "Wv": np.asarray(Wv, np.float32).astype(BF),
            "Ws": np.asarray(Ws, np.float32).astype(BF),
            "bq": np.asarray(bq, np.float32).reshape(DEPTH, C, 1),
            "bvs": bvs.transpose(1, 0, 2).reshape(2, -1).copy().astype(BF),
            "aux2": meta["aux2"][c].astype(BF),
            "idx16": meta["idx16"][c],
            "maskg": meta["maskg"][c].astype(BF),
            "aux": meta["aux"][c].reshape(128, -1).astype(BF),
            "auxf": np.ascontiguousarray(
                meta["aux"][c].reshape(128, -1, 10)[:, :, 8]).astype(np.float32),
            "invc": meta["invc"][c],
        })
    return in_maps


def assemble_output(meta, results, n_graphs=B_GRAPHS):
    gpc = meta["gpc"]
    out = np.zeros((n_graphs, DEPTH * C), np.float32)
    for c in range(NCORES):
        out[c * gpc:(c + 1) * gpc] = results[c]["out"]
    return out


_CACHE = {}


def kernel(x, edge_index, batch_ids, enc_W, enc_b, Wq, bq, Wk, bk, Wv, bv, Ws, bs):
    key = (np.asarray(x).shape, np.asarray(edge_index).tobytes()[:64],
           np.asarray(batch_ids).tobytes()[:64])
    if key not in _CACHE:
        meta = preprocess(np.asarray(edge_index), np.asarray(batch_ids))
        nc = build(meta, reps=1)
        _CACHE[key] = (meta, nc)
    meta, nc = _CACHE[key]
    in_maps = input_maps(meta, x, enc_W, enc_b, Wq, bq, Wk, bk, Wv, bv, Ws, bs)
    res = bass_utils.run_bass_kernel_spmd(nc, in_maps, core_ids=list(range(NCORES)))
    return assemble_output(meta, res.results)



# revision 14
# speedup vs baseline: 1.0714x; 1.0714x over previous
"""Trainium2 Bass kernel for nn_Encoder_71528385347709 (gnn_message_passing).

3-layer TransformerConv (heads=1) GNN encoder + per-layer global mean pool.

v2 design: nodes sharded graph-contiguously across 8 cores; per-layer halo
exchange via shared-output AllGather (DRAM); per-edge source states gathered
CHANNEL-MAJOR (transpose dma_gather, 256B rows) in 128-edge tiles grouped by
128-dst block.  The edge phase runs on the TensorEngine:
    S_t[e,d]   = hgT_t^T qtldT_b          (pairwise scores, PSUM)
    alpha_e    = sum_d S_t[e,d] * M_t[e,d]   (M = one-hot dst mask, DVE)
    ex         = exp(alpha)                   (no max-sub; |alpha|<25)
    MexT_t     = M_t * ex                      (unnormalized weights)
    AGGu_b     = sum_t hgn_t^T MexT_t          (PE, accumulated in PSUM)
    den_b      = ones^T MexT_t                 (PE row)
    h' = Wv^T (AGGu/den) + Ws^T h + bv*ind + bs*valid   (PE, PSUM)
No per-edge elementwise O(E*C) work on the vector engine.
"""
import sys
import numpy as np

sys.path.insert(0, '/opt/trn_rl_repo')

import concourse.bass as bass              # noqa: E402
import concourse.tile as tile              # noqa: E402
from concourse import bacc, mybir          # noqa: E402
from concourse.masks import make_identity  # noqa: E402
import concourse.bass_utils as bass_utils  # noqa: E402

F32 = mybir.dt.float32
BF16 = mybir.dt.bfloat16
I16 = mybir.dt.int16
ALU = mybir.AluOpType
AXL = mybir.AxisListType
ACTF = mybir.ActivationFunctionType

NCORES = 8
C = 128
DEPTH = 3
B_GRAPHS = 64
SBW = 4             # blocks per superblock
BATCH = 4           # tiles per DVE batch
CALLT = 8           # tiles per gather call
PADSLOT = 200.0     # dstslot value for dummy edge columns


# ---------------------------------------------------------------- host prep
def preprocess(edge_index, batch_ids, n_graphs=B_GRAPHS):
    src = np.asarray(edge_index[0], np.int64)
    dst = np.asarray(edge_index[1], np.int64)
    bid = np.asarray(batch_ids, np.int64)
    N = bid.shape[0]
    gpc = n_graphs // NCORES

    bounds = np.searchsorted(bid, np.arange(NCORES + 1) * gpc)
    L = np.diff(bounds)
    NB = int(np.ceil((L.max() + 1) / 128.0))
    PL = NB * 128
    NF = NCORES * PL
    A_HI = min(NF, 32768)
    B_LO = max(0, NF - 32768)

    indeg = np.bincount(dst, minlength=N)

    # --- per-core balanced bin packing of nodes into NB blocks (cap 128),
    # balancing A-edge and B-edge counts separately so tile counts stay low.
    dev_row = np.empty(N, np.int64)
    perms = []
    for c in range(NCORES):
        n0, n1 = int(bounds[c]), int(bounds[c + 1])
        nodes = np.arange(n0, n1)
        deg = indeg[n0:n1]
        order = np.argsort(-deg, kind='stable')
        bsum = np.zeros(NB)
        bcnt = np.zeros(NB, np.int64)
        assign = np.empty(n1 - n0, np.int64)
        for i in order:
            open_b = np.flatnonzero(bcnt < 128)
            j = open_b[np.argmin(bsum[open_b])]
            assign[i] = j
            bsum[j] += deg[i]
            bcnt[j] += 1
        # order bins by descending A-load so heavy bins align across cores
        border = np.argsort(-bsum, kind='stable')
        rank = np.empty(NB, np.int64)
        rank[border] = np.arange(NB)
        slot = np.zeros(NB, np.int64)
        rows = np.empty(n1 - n0, np.int64)
        for i in range(n1 - n0):
            b = rank[assign[i]]
            rows[i] = b * 128 + slot[b]
            slot[b] += 1
        dev_row[nodes] = c * PL + rows
        # perm maps padded local row -> original node (or -1)
        pfull = np.full(PL, -1, np.int64)
        pfull[rows] = nodes
        perms.append(pfull)

    sdev = dev_row[src]
    ddev = dev_row[dst]
    isA = sdev < A_HI
    c_ = ddev // PL
    jloc = ddev % PL
    b_ = jloc // 128
    p_ = jloc % 128

    # per (core, block, half) edge lists
    cnt = np.zeros((NCORES, NB, 2), np.int64)
    for half in (0, 1):
        m = isA if half == 0 else ~isA
        np.add.at(cnt[:, :, half], (c_[m], b_[m]), 1)
    TA = np.ceil(cnt[:, :, 0].max(axis=0) / 128.0).astype(np.int64)
    TB = np.ceil(cnt[:, :, 1].max(axis=0) / 128.0).astype(np.int64)
    TA = np.maximum(TA, 1)
    TB = np.maximum(TB, 1)

    # edge order: sort by (core, block, half) then fill tiles
    key = ((c_ * NB + b_) * 2 + (~isA)).astype(np.int64)
    eo = np.argsort(key, kind='stable')

    SB = int(np.ceil(NB / SBW))
    dummyA = PL - 1
    dummyB = NF - 1 - B_LO

    # build stream of tiles: block-contiguous, A tiles then B tiles
    tiles = []    # (block, half, first_of_block, last_of_block)
    for b in range(NB):
        for half in (0, 1):
            nt = int(TA[b] if half == 0 else TB[b])
            for i in range(nt):
                first = (half == 0 and i == 0)
                last = (half == 1 and i == nt - 1)
                tiles.append((b, half, first, last))
    T = len(tiles)

    # per-core idx + dstslot grids
    idxs = np.full((NCORES, T * 128), 0, np.int64)
    for t, (b, half, _, _) in enumerate(tiles):
        idxs[:, t * 128:(t + 1) * 128] = dummyA if half == 0 else dummyB
    cur = {}
    for t, (b, half, _, _) in enumerate(tiles):
        cur.setdefault((b, half), []).append(t)
    eo_src = sdev[eo]
    eo_isA = isA[eo]
    eo_c = c_[eo]
    eo_b = b_[eo]
    eo_p = p_[eo]
    dslotc = np.full((NCORES, 128, T), PADSLOT, np.float32)
    pos_in_seg = np.zeros((NCORES, NB, 2), np.int64)
    for i in range(eo.shape[0]):
        cc, bb = eo_c[i], eo_b[i]
        hh = 0 if eo_isA[i] else 1
        k = pos_in_seg[cc, bb, hh]
        pos_in_seg[cc, bb, hh] += 1
        tl = cur[(bb, hh)][k // 128]
        col = k % 128
        idxs[cc, tl * 128 + col] = eo_src[i] if hh == 0 else eo_src[i] - B_LO
        dslotc[cc, col, tl] = eo_p[i]
    assert idxs.max() <= 32767 and idxs.min() >= 0

    # gather calls: runs of <=CALLT tiles with constant half
    calls = []   # (tile0, ntiles, is_b)
    t0 = 0
    while t0 < T:
        h0 = tiles[t0][1]
        n = 1
        while (t0 + n < T and n < CALLT
               and tiles[t0 + n][1] == h0):
            n += 1
        calls.append((t0, n, h0 == 1))
        t0 += n

    # wrapped idx16 per call
    idx_cols = sum(8 * n for (_, n, _) in calls)
    idx16 = np.zeros((NCORES, 128, idx_cols), np.int16)
    off = 0
    call_meta = []
    for (ct0, n, is_b) in calls:
        ni = n * 128
        flat = idxs[:, ct0 * 128: ct0 * 128 + ni]
        wrapped = flat.reshape(NCORES, ni // 16, 16).transpose(0, 2, 1)
        idx16[:, :16, off:off + ni // 16] = wrapped.astype(np.int16)
        idx16[:, 16:, off:off + ni // 16] = np.tile(
            wrapped, (1, 7, 1)).astype(np.int16)
        call_meta.append({"t0": ct0, "ntiles": n, "is_b": is_b,
                          "idx_off": off, "ni": ni})
        off += ni // 16

    # aux tables
    auxg = np.zeros((NCORES, 128, NB, gpc), np.float32)
    auxf = np.zeros((NCORES, 128, NB), np.float32)
    masks2 = np.zeros((NCORES, 2, PL), np.float32)   # rows: indeg>0, valid
    mrow0 = np.zeros((NCORES, 1, PL), np.float32)    # indeg==0 guard
    for c in range(NCORES):
        pf = perms[c]
        valid = pf >= 0
        rows = np.arange(PL)
        gids = np.where(valid, bid[np.where(valid, pf, 0)] - c * gpc, 0)
        auxg[c, rows % 128, rows // 128, :] = 0.0
        auxg[c][rows[valid] % 128, rows[valid] // 128, gids[valid]] = 1.0
        auxf[c, rows[valid] % 128, rows[valid] // 128] = 1.0
        iv = np.where(valid, indeg[np.where(valid, pf, 0)], 0)
        masks2[c, 0, :] = np.where(valid & (iv > 0), 1.0, 0.0)
        masks2[c, 1, :] = valid.astype(np.float32)
        mrow0[c, 0, :] = 1.0 - masks2[c, 0, :]

    counts = np.maximum(np.bincount(bid, minlength=n_graphs), 1).astype(np.float32)
    invc = (1.0 / counts).reshape(NCORES, gpc, 1)
    iotaB = np.tile(np.arange(128, dtype=np.float32)[None, :], (128, 1))

    return dict(NB=NB, PL=PL, NF=NF, B_LO=B_LO, SB=SB, T=T, tiles=tiles,
                call_meta=call_meta, idx16=idx16, dslotc=dslotc,
                auxg=auxg, auxf=auxf, masks2=masks2, mrow0=mrow0, invc=invc,
                iotaB=iotaB, perms=perms, L=L, gpc=gpc, idx_cols=idx_cols)


# ---------------------------------------------------------------- device build
def build(meta, reps=1, scratch=16384, nq=4):
    NB, PL, NF, B_LO = meta["NB"], meta["PL"], meta["NF"], meta["B_LO"]
    T = meta["T"]
    tiles = meta["tiles"]
    idx_cols = meta["idx_cols"]
    gpc = meta["gpc"]
    scale = float(1.0 / np.sqrt(C))

    nc = bacc.Bacc("TRN2", target_bir_lowering=False, debug=False,
                   num_devices=NCORES, dynamic_dma_scratch_size=scratch,
                   num_swdge_queues=nq)

    xT_d = nc.dram_tensor("xT", [4, PL], BF16, kind="ExternalInput")
    encW_d = nc.dram_tensor("encW", [4, C - 4], BF16, kind="ExternalInput")
    encbr_d = nc.dram_tensor("encbr", [128, C - 4], F32, kind="ExternalInput")
    Wq_d = nc.dram_tensor("Wq", [DEPTH, C, C], BF16, kind="ExternalInput")
    WkT_d = nc.dram_tensor("WkT", [DEPTH, C, C], BF16, kind="ExternalInput")
    Wv_d = nc.dram_tensor("Wv", [DEPTH, C, C], BF16, kind="ExternalInput")
    Ws_d = nc.dram_tensor("Ws", [DEPTH, C, C], BF16, kind="ExternalInput")
    bq_d = nc.dram_tensor("bq", [DEPTH, C, 1], F32, kind="ExternalInput")
    bvsr_d = nc.dram_tensor("bvsr", [2, DEPTH * C], BF16, kind="ExternalInput")
    masks2_d = nc.dram_tensor("masks2", [2, PL], BF16, kind="ExternalInput")
    mrow0_d = nc.dram_tensor("mrow0", [1, PL], F32, kind="ExternalInput")
    idx_d = nc.dram_tensor("idx16", [128, idx_cols], I16, kind="ExternalInput")
    dslot_d = nc.dram_tensor("dslot", [128, T], BF16, kind="ExternalInput")
    iota_d = nc.dram_tensor("iotaB", [128, 128], BF16, kind="ExternalInput")
    auxg_d = nc.dram_tensor("auxg", [128, NB * gpc], BF16, kind="ExternalInput")
    auxf_d = nc.dram_tensor("auxf", [128, NB], F32, kind="ExternalInput")
    invc_d = nc.dram_tensor("invc", [gpc, 1], F32, kind="ExternalInput")
    out_d = nc.dram_tensor("out", [gpc, DEPTH * C], F32, kind="ExternalOutput")

    hf_sh = [nc.dram_tensor(f"hf{l}", [NF, C], BF16, addr_space="Shared")
             for l in range(DEPTH)]
    in_b = nc.dram_tensor("in_b", [PL, C], BF16)

    lowprec = nc.allow_low_precision(reason="bf16 edge phase, tol 2e-2")
    with tile.TileContext(nc) as tc, lowprec:
        with tc.tile_pool(name="cst", bufs=1) as cst, \
             tc.tile_pool(name="st", bufs=1) as st, \
             tc.tile_pool(name="wk", bufs=3) as wk, \
             tc.tile_pool(name="hgp", bufs=3) as hgp, \
             tc.tile_pool(name="ps", bufs=2, space="PSUM") as ps, \
             tc.tile_pool(name="psb", bufs=1, space="PSUM") as psb, \
             tc.tile_pool(name="pacc", bufs=2, space="PSUM") as pacc, \
             tc.tile_pool(name="pden", bufs=2, space="PSUM") as pden, \
             tc.tile_pool(name="psp", bufs=1, space="PSUM") as psp:

            # ---- constants
            xT = cst.tile([4, PL], BF16)
            nc.sync.dma_start(out=xT[:], in_=xT_d[:])
            encW = cst.tile([4, C - 4], BF16)
            encbr = cst.tile([128, C - 4], F32)
            nc.sync.dma_start(out=encW[:], in_=encW_d[:])
            nc.sync.dma_start(out=encbr[:], in_=encbr_d[:])
            Wq = cst.tile([C, DEPTH, C], BF16)
            WkT = cst.tile([C, DEPTH, C], BF16)
            Wv = cst.tile([C, DEPTH, C], BF16)
            Ws = cst.tile([C, DEPTH, C], BF16)
            bq = cst.tile([C, DEPTH, 1], F32)
            for (t_, d_) in ((Wq, Wq_d), (WkT, WkT_d), (Wv, Wv_d), (Ws, Ws_d),
                             (bq, bq_d)):
                nc.sync.dma_start(out=t_[:], in_=d_[:].rearrange("l a b -> a l b"))
            bvsr = cst.tile([2, DEPTH, C], BF16)
            nc.sync.dma_start(out=bvsr[:],
                              in_=bvsr_d[:].rearrange("t (l c) -> t l c", l=DEPTH))
            masks2 = cst.tile([2, PL], BF16)
            nc.sync.dma_start(out=masks2[:], in_=masks2_d[:])
            mrow0 = cst.tile([1, PL], F32)
            nc.sync.dma_start(out=mrow0[:], in_=mrow0_d[:])
            idx16 = cst.tile([128, idx_cols], I16)
            nc.sync.dma_start(out=idx16[:], in_=idx_d[:])
            dslot = cst.tile([128, T], BF16)
            nc.sync.dma_start(out=dslot[:], in_=dslot_d[:])
            iotaB = cst.tile([128, 128], BF16)
            nc.sync.dma_start(out=iotaB[:], in_=iota_d[:])
            auxg = cst.tile([128, NB, gpc], BF16)
            nc.sync.dma_start(out=auxg[:],
                              in_=auxg_d[:].rearrange("p (b g) -> p b g", b=NB))
            auxf = cst.tile([128, NB, 1], F32)
            nc.sync.dma_start(out=auxf[:].rearrange("p b o -> p (b o)"),
                              in_=auxf_d[:])
            invc = cst.tile([gpc, 1], F32)
            nc.sync.dma_start(out=invc[:], in_=invc_d[:])
            ident = cst.tile([128, 128], BF16)
            make_identity(nc, ident[:])
            onesc = cst.tile([128, 1], BF16)
            nc.vector.memset(onesc[:], 1.0)

            # ---- persistent state
            hT = st.tile([128, PL], BF16)
            hnm = st.tile([128, NB, C], BF16)
            qtldT = st.tile([128, PL], BF16)
            alphag = st.tile([128, T], F32)
            exg = st.tile([128, T], BF16)
            outp = st.tile([gpc, DEPTH * C], F32)

            for _rep in range(reps):
                # ===== h0 = [x, x@encW + encb] node-major -> hnm, hT
                for b in range(NB):
                    pbt = ps.tile([128, 4, 128], F32, space="PSUM", tag="S",
                                  name="pbt")
                    pb = pbt[:, 0, :]
                    nc.tensor.matmul(out=pb[:, 0:C - 4],
                                     lhsT=xT[:, b * 128:(b + 1) * 128],
                                     rhs=encW[:], start=True, stop=True)
                    nc.vector.tensor_tensor(
                        out=hnm[:, b, 4:C], in0=pb[:, 0:C - 4],
                        in1=encbr[:], op=ALU.add)
                    ptrt = psb.tile([128, 4, 128], BF16, space="PSUM", tag="T",
                                    name="ptrt")
                    ptr = ptrt[:, 0, :]
                    nc.tensor.transpose(out=ptr[:, 0:4],
                                        in_=xT[:, b * 128:(b + 1) * 128],
                                        identity=ident[0:4, 0:4])
                    nc.vector.tensor_copy(out=hnm[:, b, 0:4], in_=ptr[:, 0:4])
                    nc.vector.tensor_scalar(out=hnm[:, b, :], in0=hnm[:, b, :],
                                            scalar1=auxf[:, b, 0:1], scalar2=None,
                                            op0=ALU.mult)
                    ptr2t = psb.tile([128, 4, 128], BF16, space="PSUM", tag="T",
                                     name="ptr2t")
                    ptr2 = ptr2t[:, 0, :]
                    nc.tensor.transpose(out=ptr2[:], in_=hnm[:, b, :],
                                        identity=ident[:])
                    nc.scalar.copy(out=hT[:, b * 128:(b + 1) * 128],
                                   in_=ptr2[:])

                for l in range(DEPTH):
                    # ===== halo exchange (AllGather to shared DRAM)
                    nc.sync.dma_start(
                        out=in_b[:].rearrange("(b p) c -> p b c", p=128),
                        in_=hnm[:])
                    nc.gpsimd.collective_compute(
                        "AllGather", ALU.bypass,
                        replica_groups=[list(range(NCORES))],
                        ins=[in_b[:].opt()], outs=[hf_sh[l][:].opt()])
                    hf = hf_sh[l]

                    # ===== node phase: qtldT = scale * Wk (Wq^T h)^T, ch-major
                    for n0 in range(0, PL, 512):
                        nw = min(512, PL - n0)
                        pqt = ps.tile([128, 4, 128], F32, space="PSUM",
                                      tag="S", name="pqt")
                        pq = pqt[:].rearrange("c a b -> c (a b)")
                        nc.tensor.matmul(out=pq[:, 0:nw], lhsT=Wq[:, l, :],
                                         rhs=hT[:, n0:n0 + nw],
                                         start=True, stop=True)
                        qt = wk.tile([C, 512], BF16, tag="qt")
                        nc.vector.tensor_scalar(out=qt[:, 0:nw], in0=pq[:, 0:nw],
                                                scalar1=bq[:, l, 0:1],
                                                scalar2=None, op0=ALU.add)
                        pq2t = ps.tile([128, 4, 128], F32, space="PSUM",
                                       tag="S", name="pq2t")
                        pq2 = pq2t[:].rearrange("c a b -> c (a b)")
                        nc.tensor.matmul(out=pq2[:, 0:nw], lhsT=WkT[:, l, :],
                                         rhs=qt[:, 0:nw], start=True, stop=True)
                        nc.vector.tensor_scalar(out=qtldT[:, n0:n0 + nw],
                                                in0=pq2[:, 0:nw], scalar1=scale,
                                                scalar2=None, op0=ALU.mult)

                    # ===== edge phase
                    ppool = psp.tile([gpc, C], F32, space="PSUM", tag="pp")
                    aggP = {}
                    denP = {}
                    qrr = 0
                    for cm in meta["call_meta"]:
                        ct0, ntl = cm["t0"], cm["ntiles"]
                        hg = hgp.tile([128, CALLT * 128], BF16, tag="hg")
                        src_ap = hf[B_LO:, :] if cm["is_b"] else \
                            hf[0:min(NF, 32768), :]
                        nc.gpsimd.dma_gather(
                            out_ap=hg[:, 0:ntl * 128].rearrange(
                                "p (o n) -> p o n", o=1),
                            in_ap=src_ap,
                            idxs_ap=idx16[:, cm["idx_off"]:
                                          cm["idx_off"] + cm["ni"] // 16],
                            num_idxs=cm["ni"], num_idxs_reg=cm["ni"],
                            elem_size=C, transpose=True,
                            queue_num=qrr % nq)
                        qrr += 1
                        for bt0 in range(0, ntl, BATCH):
                            bn = min(BATCH, ntl - bt0)
                            t0 = ct0 + bt0
                            Sp = ps.tile([128, BATCH, 128], F32, space="PSUM",
                                         tag="S")
                            Tp = psb.tile([128, BATCH, 128], BF16, space="PSUM",
                                          tag="T")
                            for j in range(bn):
                                t = t0 + j
                                blk = tiles[t][0]
                                hsl = hg[:, (bt0 + j) * 128:(bt0 + j + 1) * 128]
                                nc.tensor.matmul(
                                    out=Sp[:, j, :], lhsT=hsl,
                                    rhs=qtldT[:, blk * 128:(blk + 1) * 128],
                                    start=True, stop=True)
                                nc.tensor.transpose(out=Tp[:, j, :], in_=hsl,
                                                    identity=ident[:])
                            hgn = wk.tile([128, BATCH, 128], BF16, tag="hgn")
                            nc.vector.tensor_copy(out=hgn[:, 0:bn, :],
                                                  in_=Tp[:, 0:bn, :])
                            mex = wk.tile([128, BATCH, 128], BF16, tag="mex")
                            nc.vector.tensor_tensor(
                                out=mex[:, 0:bn, :],
                                in0=iotaB[:].unsqueeze(1)
                                .to_broadcast([128, bn, 128]),
                                in1=dslot[:, t0:t0 + bn].unsqueeze(2)
                                .to_broadcast([128, bn, 128]),
                                op=ALU.is_equal)
                            nc.vector.tensor_tensor(
                                out=Sp[:, 0:bn, :], in0=Sp[:, 0:bn, :],
                                in1=mex[:, 0:bn, :], op=ALU.mult)
                            nc.vector.tensor_reduce(
                                out=alphag[:, t0:t0 + bn], in_=Sp[:, 0:bn, :],
                                axis=AXL.X, op=ALU.add)
                            nc.scalar.activation(
                                out=exg[:, t0:t0 + bn],
                                in_=alphag[:, t0:t0 + bn],
                                func=ACTF.Exp, scale=1.0)
                            nc.vector.tensor_tensor(
                                out=mex[:, 0:bn, :], in0=mex[:, 0:bn, :],
                                in1=exg[:, t0:t0 + bn].unsqueeze(2)
                                .to_broadcast([128, bn, 128]),
                                op=ALU.mult)
                            for j in range(bn):
                                t = t0 + j
                                blk, half, first, last = tiles[t]
                                if first:
                                    agg_t = pacc.tile(
                                        [128, 128], F32, space="PSUM",
                                        tag="agg", name="agg_t")
                                    den_t = pden.tile(
                                        [1, 128], F32, space="PSUM",
                                        tag="den", name="den_t")
                                    aggP["t"] = agg_t
                                    denP["t"] = den_t
                                nc.tensor.matmul(
                                    out=aggP["t"][:],
                                    lhsT=hgn[:, j, :], rhs=mex[:, j, :],
                                    start=first, stop=last)
                                nc.tensor.matmul(
                                    out=denP["t"][:],
                                    lhsT=onesc[:], rhs=mex[:, j, :],
                                    start=first, stop=last)
                                if last:
                                    b = blk
                                    bsl = slice(b * 128, (b + 1) * 128)
                                    drow = wk.tile([1, 128], F32, tag="dr")
                                    nc.vector.tensor_tensor(
                                        out=drow[:], in0=denP["t"][:],
                                        in1=mrow0[0:1, bsl], op=ALU.add)
                                    rrow = wk.tile([1, 128], F32, tag="rr")
                                    nc.vector.reciprocal(out=rrow[:],
                                                         in_=drow[:])
                                    rfull = wk.tile([128, 128], F32, tag="rf")
                                    nc.gpsimd.partition_broadcast(
                                        out_ap=rfull[:], in_ap=rrow[:])
                                    aggn = wk.tile([128, 128], BF16, tag="an")
                                    nc.vector.tensor_tensor(
                                        out=aggn[:], in0=aggP["t"][:],
                                        in1=rfull[:], op=ALU.mult)
                                    upt = ps.tile([128, 4, 128], F32,
                                                  space="PSUM", tag="S",
                                                  name="upt")
                                    up = upt[:, 0, :]
                                    nc.tensor.matmul(out=up[:], lhsT=Wv[:, l, :],
                                                     rhs=aggn[:],
                                                     start=True, stop=False)
                                    nc.tensor.matmul(out=up[:], lhsT=Ws[:, l, :],
                                                     rhs=hT[:, bsl],
                                                     start=False, stop=False)
                                    nc.tensor.matmul(out=up[:],
                                                     lhsT=bvsr[:, l, :],
                                                     rhs=masks2[0:2, bsl],
                                                     start=False, stop=True)
                                    nc.vector.tensor_copy(out=hT[:, bsl],
                                                          in_=up[:])
                                    trt = psb.tile([128, 4, 128], BF16,
                                                   space="PSUM", tag="T",
                                                   name="trt")
                                    tr = trt[:, 0, :]
                                    nc.tensor.transpose(out=tr[:],
                                                        in_=hT[:, bsl],
                                                        identity=ident[:])
                                    nc.scalar.copy(out=hnm[:, b, :], in_=tr[:])
                                    nc.tensor.matmul(
                                        out=ppool[:], lhsT=auxg[:, b, :],
                                        rhs=hnm[:, b, :], start=(b == 0),
                                        stop=(b == NB - 1))
                    nc.vector.tensor_scalar(out=outp[:, l * C:(l + 1) * C],
                                            in0=ppool[:], scalar1=invc[:, 0:1],
                                            scalar2=None, op0=ALU.mult)

            nc.sync.dma_start(out=out_d[:], in_=outp[:])
    nc.compile()
    return nc


# ---------------------------------------------------------------- input maps
def input_maps(meta, x, enc_W, enc_b, Wq, bq, Wk, bk, Wv, bv, Ws, bs):
    PL = meta["PL"]
    NB = meta["NB"]
    gpc = meta["gpc"]
    BF = mybir.dt.np(mybir.dt.bfloat16)
    in_maps = []
    WkT = np.ascontiguousarray(np.transpose(np.asarray(Wk, np.float32), (0, 2, 1)))
    bvs = np.stack([np.asarray(bv, np.float32),
                    np.asarray(bs, np.float32)], axis=1)  # [C?, 2, ...]
    for c in range(NCORES):
        pf = meta["perms"][c]
        xp = np.zeros((PL, 4), np.float32)
        valid = pf >= 0
        xp[valid] = np.asarray(x, np.float32)[pf[valid]]
        in_maps.append({
            "xT": np.ascontiguousarray(xp.T).astype(BF),
            "encW": np.asarray(enc_W, np.float32).astype(BF),
            "encbr": np.tile(np.asarray(enc_b, np.float32).reshape(1, -1),
                             (128, 1)),
            "Wq": np.asarray(Wq, np.float32).astype(BF),
            "WkT": WkT.astype(BF),
            "Wv": np.asarray(Wv, np.float32).astype(BF),
            "Ws": np.asarray(Ws, np.float32).astype(BF),
            "bq": np.asarray(bq, np.float32).reshape(DEPTH, C, 1),
            "bvsr": bvs.transpose(1, 0, 2).reshape(2, -1).copy().astype(BF),
            "masks2": meta["masks2"][c].astype(BF),
            "mrow0": meta["mrow0"][c],
            "idx16": meta["idx16"][c],
            "dslot": meta["dslotc"][c].astype(BF),
            "iotaB": meta["iotaB"].astype(BF),
            "auxg": meta["auxg"][c].reshape(128, -1).astype(BF),
            "auxf": meta["auxf"][c],
            "invc": meta["invc"][c],
        })
    return in_maps


def assemble_output(meta, results, n_graphs=B_GRAPHS):
    gpc = meta["gpc"]
    out = np.zeros((n_graphs, DEPTH * C), np.float32)
    for c in range(NCORES):
        out[c * gpc:(c + 1) * gpc] = results[c]["out"]
    return out


_CACHE = {}


def kernel(x, edge_index, batch_ids, enc_W, enc_b, Wq, bq, Wk, bk, Wv, bv, Ws, bs):
    key = (np.asarray(x).shape, np.asarray(edge_index).tobytes()[:64],
           np.asarray(batch_ids).tobytes()[:64])
    if key not in _CACHE:
        meta = preprocess(np.asarray(edge_index), np.asarray(batch_ids))
        nc = build(meta, reps=1)
        _CACHE[key] = (meta, nc)
    meta, nc = _CACHE[key]
    in_maps = input_maps(meta, x, enc_W, enc_b, Wq, bq, Wk, bk, Wv, bv, Ws, bs)
    res = bass_utils.run_bass_kernel_spmd(nc, in_maps, core_ids=list(range(NCORES)))
    return assemble_output(meta, res.results)
